# revision 31
# baseline (speedup 1.0000x reference)
"""Trainium2 kernel for nn_Circuit_28123445854302.

24-wire statevector circuit (depth-4 brickwork, 46 two-qubit gates) applied to
a product state.  Strategy:

The statevector is sharded over its 3 leading wire axes across the 8 cores
(state-index sharding, as hinted).  Rather than streaming the 64 MB state
through every gate, we exploit the circuit's 1-D locality: across the middle
wire cut (12|12) only the gates that straddle the cut can raise the Schmidt
rank, so the final state factors EXACTLY as

    psi[left, right] = sum_r A[r, left] * B[r, right]        (rank R, tiny)

For the brickwork circuit R == 16.  A and B (R x 4096) are computed exactly on
the host in float64 with negligible cost (all tensors are O(R * 2^12)); every
element of the 2^24 statevector is then produced ON DEVICE by a K=R matmul:

    core c:  out[512, 4096] = A[:, c*512:(c+1)*512].T @ B        (rows = left
             indices with leading-3-wire bits == c, i.e. the core's shard)

Each core writes its contiguous shard of the output; the host gather is a
plain concatenate.

Device pipeline (cost-model-guided):
  * The kernel is output-DMA bound: the out shard per core is the only large
    HBM traffic.  Emitting it in bfloat16 (upcast to f32 on the host during
    the gather) halves that traffic; the bf16 rounding of the OUTPUT is the
    only error introduced (~1e-3 rel, vs the 2e-2 gate).
  * The matmul keeps A to ~f32 accuracy via a "split2" trick: A is split as
    hi=bf16(A), lo=bf16(A-hi) and the rank-R contraction is widened to 2R
    rows so one bf16 matmul accumulates (hi+lo) @ bf16(B) in fp32 PSUM.
    K does not affect TensorE time (cost = N output cycles), so the extra
    rows only cost input-DMA bytes.
  * Column-group software pipeline: matmul (PE) -> PSUM->SBUF cast-copy
    (alternating ScalarE/VectorE so neither is the straggler) -> streaming
    HWDGE DMA of each bf16 group to HBM.  Groups are sized small at the head
    (to start the output-DMA chain early) and 1024 wide in steady state
    (DMA transfer per group ~= copy time of the idle engine).

If a (hypothetical) non-local gate list makes the cut rank explode, we fall
back to an exact dense numpy simulation (same semantics as the reference).
"""

import numpy as np

_N_WIRES = 24
_CUT = 12
_HALF = 1 << _CUT          # 4096
_N_CORES = 8
_ROWS_PER_CORE = _HALF // _N_CORES   # 512
_MAX_RANK = 512


# ----------------------------------------------------------------------------
# Host-side exact middle-cut factorization (all tiny tensors, float64)
# ----------------------------------------------------------------------------

def _apply_2q(M, g, w0, w1, nloc):
    """Apply gate g[i0,o0,i1,o1] on local wires w0,w1 of every row of
    M (R, 2**nloc).  Matches reference: tensordot + moveaxis."""
    R = M.shape[0]
    T = M.reshape((R,) + (2,) * nloc)
    src = [4] + [0 if k == w0 else (2 if k == w1 else 5 + k) for k in range(nloc)]
    dst = [4] + [1 if k == w0 else (3 if k == w1 else 5 + k) for k in range(nloc)]
    return np.einsum(g, [0, 1, 2, 3], T, src, dst).reshape(R, -1)


def _apply_1q(M, P, w, nloc):
    """Apply P[i,o] on local wire w of every row of M (R, 2**nloc)."""
    R = M.shape[0]
    T = M.reshape((R,) + (2,) * nloc)
    src = [4] + [0 if t == w else 5 + t for t in range(nloc)]
    dst = [4] + [1 if t == w else 5 + t for t in range(nloc)]
    return np.einsum(P, [0, 1], T, src, dst).reshape(R, -1)


def _build_factors(states, gates, gate_wires):
    """psi = A.T @ B with A, B (R, 4096) float64, or None if rank > _MAX_RANK."""
    states = np.asarray(states, dtype=np.float64)
    gates = np.asarray(gates, dtype=np.float64)
    wires = np.asarray(gate_wires)
    NR = _N_WIRES - _CUT

    def outer(lo, hi):
        v = states[lo]
        for w in range(lo + 1, hi):
            v = np.kron(v, states[w])
        return v

    A = outer(0, _CUT)[None, :].copy()
    B = outer(_CUT, _N_WIRES)[None, :].copy()

    for gi in range(gates.shape[0]):
        w0, w1 = int(wires[gi, 0]), int(wires[gi, 1])
        g = gates[gi]
        if w0 == w1:
            return None  # ill-defined for the reference too; bail out
        if w0 > w1:
            g = np.transpose(g, (2, 3, 0, 1))
            w0, w1 = w1, w0
        if w1 < _CUT:
            A = _apply_2q(A, g, w0, w1, _CUT)
        elif w0 >= _CUT:
            B = _apply_2q(B, g, w0 - _CUT, w1 - _CUT, NR)
        else:
            # Gate straddles the cut: operator-Schmidt split (rank <= 4).
            M4 = g.reshape(4, 4)  # rows (i0,o0) act left, cols (i1,o1) act right
            U, s, Vt = np.linalg.svd(M4)
            rank = max(1, int((s > s[0] * 1e-14).sum()))
            newA, newB = [], []
            for k in range(rank):
                P = (U[:, k] * s[k]).reshape(2, 2)
                Q = Vt[k].reshape(2, 2)
                newA.append(_apply_1q(A, P, w0, _CUT))
                newB.append(_apply_1q(B, Q, w1 - _CUT, NR))
            A = np.concatenate(newA, 0)
            B = np.concatenate(newB, 0)
            # Exact recompression (drops only numerically-zero directions).
            if A.shape[0] > 4:
                qa, ra = np.linalg.qr(A.T)
                qb, rb = np.linalg.qr(B.T)
                u, sv, vt = np.linalg.svd(ra @ rb.T)
                keep = max(1, int((sv > (sv[0] if sv.size else 1.0) * 1e-13).sum()))
                A = (qa @ (u[:, :keep] * sv[:keep])).T
                B = vt[:keep] @ qb.T
            if A.shape[0] > _MAX_RANK:
                return None
    return A, B


# ----------------------------------------------------------------------------
# Dense fallback (exact reference semantics in numpy) — only used if the gate
# list is so non-local that the middle-cut rank explodes.
# ----------------------------------------------------------------------------

def _dense_fallback(states, gates, gate_wires):
    states = np.asarray(states, dtype=np.float32)
    gates = np.asarray(gates, dtype=np.float32)
    wires = np.asarray(gate_wires)
    psi = states[0]
    for w in range(1, _N_WIRES):
        psi = np.multiply.outer(psi, states[w])
    for g in range(gates.shape[0]):
        w0, w1 = int(wires[g, 0]), int(wires[g, 1])
        psi = np.tensordot(gates[g], psi, axes=[[0, 2], [w0, w1]])
        psi = np.moveaxis(psi, (0, 1), (w0, w1))
    return psi


# ----------------------------------------------------------------------------
# Device kernel: out[512, 4096] (bf16) = lhsT.T @ rhs  per core, pipelined
# ----------------------------------------------------------------------------

_COMPILED = {}


_HEAD_WIDTHS = [256, 256, 512, 1024, 1024, 1024]
_WARMUP_MMS = 0


def _default_schedule(head=None, split=None, split_m0_only=False):
    """List of (m, c0, c1, copies) groups in issue order; `copies` is a list
    of (width, engine) sub-copies covering [c0, c1) and each group gets one
    output DMA (waiting on all its sub-copies via subtile deps).

    Small head groups prime the output-DMA chain; 1024-wide steady-state
    groups keep per-instruction overheads amortized.  DVE goes first: the
    scheduler estimates ScalarE's first copy late (one-time activation-table
    load), which would head-of-line-block the first output DMA on the SP
    sequencer."""
    sched = []
    engs = ["dve", "act"]
    gi = 0
    for m in range(_ROWS_PER_CORE // 128):
        widths = (head or _HEAD_WIDTHS) if m == 0 else [1024, 1024, 1024, 1024]
        c = 0
        for w in widths:
            if split and w >= 512 and (not split_m0_only or m == 0):
                # parallel split across both engines, sized to their speeds
                wa = min(w - 64, max(64, int(round(w * split / 64)) * 64))
                copies = [(wa, "act"), (w - wa, "dve")]
            else:
                copies = [(w, engs[gi % 2])]
            sched.append((m, c, c + w, copies))
            c += w
            gi += 1
        assert c == _HALF
    return sched


def _build_nc2(K, in_chunks=3, head=None, warmup=None, split=None,
               pool_dma=None, split_m0_only=False, in_splits=None,
               sched=None, ps_bufs=None):
    """Bass module: fact [K, 512+4096] bf16 -> out [512, 4096] bf16.

    fact columns 0:512 hold this core's lhsT block (A columns of its row
    shard); columns 512: hold rhs (= B, shared by all cores).
    """
    import concourse.bass as bass
    import concourse.tile as tile
    from concourse import bacc, mybir

    nc = bacc.Bacc(
        "TRN2",
        target_bir_lowering=False,
        debug=False,
        enable_asserts=False,
        num_devices=_N_CORES,
    )
    dt32 = mybir.dt.float32
    bf16 = mybir.dt.bfloat16
    CW = _ROWS_PER_CORE + _HALF   # 4608 packed input columns
    fact = nc.dram_tensor("fact", [K, CW], bf16, kind="ExternalInput").ap()
    out = nc.dram_tensor("out", [_ROWS_PER_CORE, _HALF], bf16,
                         kind="ExternalOutput").ap()

    if sched is None:
        sched = _default_schedule(head, split, split_m0_only)
    if warmup is None:
        warmup = _WARMUP_MMS
    # PSUM budget: 8 banks of 2 KB/partition.  Groups <=512 wide share the
    # one-bank "b1" tag (2 bufs); 1024-wide groups use two-bank tiles
    # (3 bufs) -> 2 + 6 = 8 banks.
    if ps_bufs is None:
        ps_bufs = {512: 2, 1024: 3}

    with tile.TileContext(nc) as tc:
        with (
            tc.tile_pool(name="const", bufs=1) as cpool,
            tc.tile_pool(name="ps", bufs=1, space=bass.MemorySpace.PSUM) as ppool,
            tc.tile_pool(name="outs", bufs=1) as opool,
        ):
            fact_sb = cpool.tile([K, CW], bf16)
            # Head chunk covers all lhsT columns + the first rhs group(s) so
            # the pipeline can start as soon as it lands; the rest streams in
            # behind it.
            if in_splits is not None:
                splits = list(in_splits)
            else:
                splits = [0, 1024]
                rem = CW - 1024
                step = (rem + (in_chunks - 1) - 1) // max(1, in_chunks - 1)
                c = 1024
                while c < CW:
                    c = min(CW, c + step)
                    splits.append(c)
            for a, b in zip(splits[:-1], splits[1:]):
                nc.sync.dma_start(fact_sb[:, a:b], fact[:, a:b])

            # Static staging: one full-width bf16 tile per m-chunk (32 KB per
            # partition total) so no copy ever waits on an output-DMA
            # completion to recycle a buffer.
            stage = [
                opool.tile([128, _HALF], bf16, tag=f"m{m}", bufs=1,
                           name=f"stage{m}")
                for m in range(_ROWS_PER_CORE // 128)
            ]

            if warmup:
                # Ramp the TensorE p-state during the input-DMA latency with
                # dummy matmuls on a zeroed scratch tile (results discarded;
                # the first real matmul overwrites the bank with start=True).
                warm = cpool.tile([128, 256], bf16, tag="warm", bufs=1,
                                  name="warm")
                nc.any.memset(warm[:], 0)
                wps = ppool.tile([128, 512], dt32, tag="b1",
                                 bufs=ps_bufs[512], name="wps")
                for _ in range(warmup):
                    nc.tensor.matmul(wps[:, :256], warm[:, :128], warm[:],
                                     start=True, stop=True)

            for m, c0, c1, copies in sched:
                W = c1 - c0
                banks = (W + 511) // 512
                psb = ppool.tile([128, 512 * banks], dt32, tag=f"b{banks}",
                                 bufs=ps_bufs[512 * banks], name="psb")
                ps = psb[:, :W]
                n_kc = (K + 127) // 128
                for j in range(0, W, 512):
                    w = min(512, W - j)
                    for kc in range(n_kc):
                        k0, k1 = kc * 128, min(K, (kc + 1) * 128)
                        nc.tensor.matmul(
                            ps[:, j:j + w],
                            fact_sb[k0:k1, m * 128:(m + 1) * 128],
                            fact_sb[k0:k1, _ROWS_PER_CORE + c0 + j:
                                    _ROWS_PER_CORE + c0 + j + w],
                            start=(kc == 0), stop=(kc == n_kc - 1),
                        )
                x = 0
                for w, eng in copies:
                    ot = stage[m][:, c0 + x:c0 + x + w]
                    if eng == "act":
                        nc.scalar.copy(ot, ps[:, x:x + w])
                    else:
                        nc.vector.tensor_copy(ot, ps[:, x:x + w])
                    x += w
                assert x == W
                if pool_dma and (m, c0) in pool_dma:
                    nc.gpsimd.dma_start(out[m * 128:(m + 1) * 128, c0:c1],
                                        stage[m][:, c0:c1])
                else:
                    nc.sync.dma_start(out[m * 128:(m + 1) * 128, c0:c1],
                                      stage[m][:, c0:c1])
    nc.compile()
    return nc


def _get_nc(K):
    if K not in _COMPILED:
        _COMPILED[K] = _build_nc2(K)
    return _COMPILED[K]


def _pack_factors(A, B):
    """f64 factors (R, 4096) -> bf16 K-stacked factors (2R, 4096).

    "split2": A is represented to ~f32 accuracy as Ah + Al (hi/lo bf16
    pair); B is plain bf16.  The rank-2R contraction computes
    (Ah + Al) @ bf16(B), so the end-to-end error is just the bf16
    rounding of B plus the bf16 rounding of the OUTPUT (~2.5e-3 rel
    total) -- far inside the 2e-2 gate.  K does not affect TensorE time,
    but smaller K shrinks the input DMA."""
    import ml_dtypes
    bf = ml_dtypes.bfloat16
    Ah = A.astype(bf)
    Al = (A - Ah.astype(np.float64)).astype(bf)
    Bh = B.astype(bf)
    Ap = np.concatenate([Ah, Al], axis=0)
    Bp = np.concatenate([Bh, Bh], axis=0)
    return Ap, Bp


def _make_in_maps(Ap, Bp):
    """Pack per-core inputs: fact = [lhsT shard | rhs] (K, 4608) bf16."""
    in_maps = []
    for c in range(_N_CORES):
        shard = Ap[:, c * _ROWS_PER_CORE:(c + 1) * _ROWS_PER_CORE]
        fact = np.concatenate([shard, Bp], axis=1)
        in_maps.append({"fact": np.ascontiguousarray(fact)})
    return in_maps


def _run_device(A, B, trace=False):
    """A, B: (R, 4096) float64 factors.  Returns (psi_flat f32, results)."""
    from concourse.bass_utils import run_bass_kernel_spmd

    Ap, Bp = _pack_factors(A, B)
    nc = _get_nc(Ap.shape[0])
    in_maps = _make_in_maps(Ap, Bp)
    res = run_bass_kernel_spmd(
        nc, in_maps, core_ids=list(range(_N_CORES)), trace=trace
    )
    flat = np.concatenate(
        [r["out"].astype(np.float32).reshape(-1) for r in res.results]
    )
    return flat, res


def kernel(states, gates, gate_wires):
    fact = _build_factors(states, gates, gate_wires)
    # 2R rows must fit the 128-partition SBUF input tile; exotic gate lists
    # that blow up the cut rank take the exact dense path instead.
    if fact is None or 2 * fact[0].shape[0] > 128:
        return _dense_fallback(states, gates, gate_wires)
    A, B = fact
    flat, _ = _run_device(A, B)
    return flat.reshape((2,) * _N_WIRES)


# revision 32
# speedup vs baseline: 1.0030x; 1.0030x over previous
"""Trainium2 kernel for nn_Circuit_28123445854302.

24-wire statevector circuit (depth-4 brickwork, 46 two-qubit gates) applied to
a product state.  Strategy:

The statevector is sharded over its 3 leading wire axes across the 8 cores
(state-index sharding, as hinted).  Rather than streaming the 64 MB state
through every gate, we exploit the circuit's 1-D locality: across the middle
wire cut (12|12) only the gates that straddle the cut can raise the Schmidt
rank, so the final state factors EXACTLY as

    psi[left, right] = sum_r A[r, left] * B[r, right]        (rank R, tiny)

For the brickwork circuit R == 16.  A and B (R x 4096) are computed exactly on
the host in float64 with negligible cost (all tensors are O(R * 2^12)); every
element of the 2^24 statevector is then produced ON DEVICE by a K=R matmul:

    core c:  out[512, 4096] = A[:, c*512:(c+1)*512].T @ B        (rows = left
             indices with leading-3-wire bits == c, i.e. the core's shard)

Each core writes its contiguous shard of the output; the host gather is a
plain concatenate.

Device pipeline (cost-model-guided):
  * The kernel is output-DMA bound: the out shard per core is the only large
    HBM traffic.  Emitting it in bfloat16 (upcast to f32 on the host during
    the gather) halves that traffic; the bf16 rounding of the OUTPUT is the
    only error introduced (~1e-3 rel, vs the 2e-2 gate).
  * The matmul keeps A to ~f32 accuracy via a "split2" trick: A is split as
    hi=bf16(A), lo=bf16(A-hi) and the rank-R contraction is widened to 2R
    rows so one bf16 matmul accumulates (hi+lo) @ bf16(B) in fp32 PSUM.
    K does not affect TensorE time (cost = N output cycles), so the extra
    rows only cost input-DMA bytes.
  * Column-group software pipeline: matmul (PE) -> PSUM->SBUF cast-copy
    (alternating ScalarE/VectorE so neither is the straggler) -> streaming
    HWDGE DMA of each bf16 group to HBM.  Groups are sized small at the head
    (to start the output-DMA chain early) and 1024 wide in steady state
    (DMA transfer per group ~= copy time of the idle engine).

If a (hypothetical) non-local gate list makes the cut rank explode, we fall
back to an exact dense numpy simulation (same semantics as the reference).
"""

import numpy as np

_N_WIRES = 24
_CUT = 12
_HALF = 1 << _CUT          # 4096
_N_CORES = 8
_ROWS_PER_CORE = _HALF // _N_CORES   # 512
_MAX_RANK = 512


# ----------------------------------------------------------------------------
# Host-side exact middle-cut factorization (all tiny tensors, float64)
# ----------------------------------------------------------------------------

def _apply_2q(M, g, w0, w1, nloc):
    """Apply gate g[i0,o0,i1,o1] on local wires w0,w1 of every row of
    M (R, 2**nloc).  Matches reference: tensordot + moveaxis."""
    R = M.shape[0]
    T = M.reshape((R,) + (2,) * nloc)
    src = [4] + [0 if k == w0 else (2 if k == w1 else 5 + k) for k in range(nloc)]
    dst = [4] + [1 if k == w0 else (3 if k == w1 else 5 + k) for k in range(nloc)]
    return np.einsum(g, [0, 1, 2, 3], T, src, dst).reshape(R, -1)


def _apply_1q(M, P, w, nloc):
    """Apply P[i,o] on local wire w of every row of M (R, 2**nloc)."""
    R = M.shape[0]
    T = M.reshape((R,) + (2,) * nloc)
    src = [4] + [0 if t == w else 5 + t for t in range(nloc)]
    dst = [4] + [1 if t == w else 5 + t for t in range(nloc)]
    return np.einsum(P, [0, 1], T, src, dst).reshape(R, -1)


def _build_factors(states, gates, gate_wires):
    """psi = A.T @ B with A, B (R, 4096) float64, or None if rank > _MAX_RANK."""
    states = np.asarray(states, dtype=np.float64)
    gates = np.asarray(gates, dtype=np.float64)
    wires = np.asarray(gate_wires)
    NR = _N_WIRES - _CUT

    def outer(lo, hi):
        v = states[lo]
        for w in range(lo + 1, hi):
            v = np.kron(v, states[w])
        return v

    A = outer(0, _CUT)[None, :].copy()
    B = outer(_CUT, _N_WIRES)[None, :].copy()

    for gi in range(gates.shape[0]):
        w0, w1 = int(wires[gi, 0]), int(wires[gi, 1])
        g = gates[gi]
        if w0 == w1:
            return None  # ill-defined for the reference too; bail out
        if w0 > w1:
            g = np.transpose(g, (2, 3, 0, 1))
            w0, w1 = w1, w0
        if w1 < _CUT:
            A = _apply_2q(A, g, w0, w1, _CUT)
        elif w0 >= _CUT:
            B = _apply_2q(B, g, w0 - _CUT, w1 - _CUT, NR)
        else:
            # Gate straddles the cut: operator-Schmidt split (rank <= 4).
            M4 = g.reshape(4, 4)  # rows (i0,o0) act left, cols (i1,o1) act right
            U, s, Vt = np.linalg.svd(M4)
            rank = max(1, int((s > s[0] * 1e-14).sum()))
            newA, newB = [], []
            for k in range(rank):
                P = (U[:, k] * s[k]).reshape(2, 2)
                Q = Vt[k].reshape(2, 2)
                newA.append(_apply_1q(A, P, w0, _CUT))
                newB.append(_apply_1q(B, Q, w1 - _CUT, NR))
            A = np.concatenate(newA, 0)
            B = np.concatenate(newB, 0)
            # Exact recompression (drops only numerically-zero directions).
            if A.shape[0] > 4:
                qa, ra = np.linalg.qr(A.T)
                qb, rb = np.linalg.qr(B.T)
                u, sv, vt = np.linalg.svd(ra @ rb.T)
                keep = max(1, int((sv > (sv[0] if sv.size else 1.0) * 1e-13).sum()))
                A = (qa @ (u[:, :keep] * sv[:keep])).T
                B = vt[:keep] @ qb.T
            if A.shape[0] > _MAX_RANK:
                return None
    return A, B


# ----------------------------------------------------------------------------
# Dense fallback (exact reference semantics in numpy) — only used if the gate
# list is so non-local that the middle-cut rank explodes.
# ----------------------------------------------------------------------------

def _dense_fallback(states, gates, gate_wires):
    states = np.asarray(states, dtype=np.float32)
    gates = np.asarray(gates, dtype=np.float32)
    wires = np.asarray(gate_wires)
    psi = states[0]
    for w in range(1, _N_WIRES):
        psi = np.multiply.outer(psi, states[w])
    for g in range(gates.shape[0]):
        w0, w1 = int(wires[g, 0]), int(wires[g, 1])
        psi = np.tensordot(gates[g], psi, axes=[[0, 2], [w0, w1]])
        psi = np.moveaxis(psi, (0, 1), (w0, w1))
    return psi


# ----------------------------------------------------------------------------
# Device kernel: out[512, 4096] (bf16) = lhsT.T @ rhs  per core, pipelined
# ----------------------------------------------------------------------------

_COMPILED = {}


_HEAD_WIDTHS = [128, 384, 576, 960, 1024, 1024]
_WARMUP_MMS = 0


def _default_schedule(head=None, split=None, split_m0_only=False):
    """List of (m, c0, c1, copies) groups in issue order; `copies` is a list
    of (width, engine) sub-copies covering [c0, c1) and each group gets one
    output DMA (waiting on all its sub-copies via subtile deps).

    Small head groups prime the output-DMA chain; 1024-wide steady-state
    groups keep per-instruction overheads amortized.  DVE goes first: the
    scheduler estimates ScalarE's first copy late (one-time activation-table
    load), which would head-of-line-block the first output DMA on the SP
    sequencer."""
    sched = []
    engs = ["dve", "act"]
    gi = 0
    for m in range(_ROWS_PER_CORE // 128):
        widths = (head or _HEAD_WIDTHS) if m == 0 else [1024, 1024, 1024, 1024]
        c = 0
        for w in widths:
            if split and w >= 512 and (not split_m0_only or m == 0):
                # parallel split across both engines, sized to their speeds
                wa = min(w - 64, max(64, int(round(w * split / 64)) * 64))
                copies = [(wa, "act"), (w - wa, "dve")]
            else:
                copies = [(w, engs[gi % 2])]
            sched.append((m, c, c + w, copies))
            c += w
            gi += 1
        assert c == _HALF
    return sched


def _build_nc2(K, in_chunks=3, head=None, warmup=None, split=None,
               pool_dma=None, split_m0_only=False, in_splits=None,
               sched=None, ps_bufs=None):
    """Bass module: fact [K, 512+4096] bf16 -> out [512, 4096] bf16.

    fact columns 0:512 hold this core's lhsT block (A columns of its row
    shard); columns 512: hold rhs (= B, shared by all cores).
    """
    import concourse.bass as bass
    import concourse.tile as tile
    from concourse import bacc, mybir

    nc = bacc.Bacc(
        "TRN2",
        target_bir_lowering=False,
        debug=False,
        enable_asserts=False,
        num_devices=_N_CORES,
    )
    dt32 = mybir.dt.float32
    bf16 = mybir.dt.bfloat16
    CW = _ROWS_PER_CORE + _HALF   # 4608 packed input columns
    fact = nc.dram_tensor("fact", [K, CW], bf16, kind="ExternalInput").ap()
    out = nc.dram_tensor("out", [_ROWS_PER_CORE, _HALF], bf16,
                         kind="ExternalOutput").ap()

    if sched is None:
        sched = _default_schedule(head, split, split_m0_only)
    if warmup is None:
        warmup = _WARMUP_MMS
    # PSUM budget: 8 banks of 2 KB/partition.  Groups <=512 wide share the
    # one-bank "b1" tag (2 bufs); 1024-wide groups use two-bank tiles
    # (3 bufs) -> 2 + 6 = 8 banks.
    if ps_bufs is None:
        ps_bufs = {512: 2, 1024: 3}

    with tile.TileContext(nc) as tc:
        with (
            tc.tile_pool(name="const", bufs=1) as cpool,
            tc.tile_pool(name="ps", bufs=1, space=bass.MemorySpace.PSUM) as ppool,
            tc.tile_pool(name="outs", bufs=1) as opool,
        ):
            fact_sb = cpool.tile([K, CW], bf16)
            # Head chunk covers all lhsT columns + the first rhs group(s) so
            # the pipeline can start as soon as it lands; the rest streams in
            # behind it.
            if in_splits is not None:
                splits = list(in_splits)
            else:
                splits = [0, 1024]
                rem = CW - 1024
                step = (rem + (in_chunks - 1) - 1) // max(1, in_chunks - 1)
                c = 1024
                while c < CW:
                    c = min(CW, c + step)
                    splits.append(c)
            for a, b in zip(splits[:-1], splits[1:]):
                nc.sync.dma_start(fact_sb[:, a:b], fact[:, a:b])

            # Static staging: one full-width bf16 tile per m-chunk (32 KB per
            # partition total) so no copy ever waits on an output-DMA
            # completion to recycle a buffer.
            stage = [
                opool.tile([128, _HALF], bf16, tag=f"m{m}", bufs=1,
                           name=f"stage{m}")
                for m in range(_ROWS_PER_CORE // 128)
            ]

            if warmup:
                # Ramp the TensorE p-state during the input-DMA latency with
                # dummy matmuls on a zeroed scratch tile (results discarded;
                # the first real matmul overwrites the bank with start=True).
                warm = cpool.tile([128, 256], bf16, tag="warm", bufs=1,
                                  name="warm")
                nc.any.memset(warm[:], 0)
                wps = ppool.tile([128, 512], dt32, tag="b1",
                                 bufs=ps_bufs[512], name="wps")
                for _ in range(warmup):
                    nc.tensor.matmul(wps[:, :256], warm[:, :128], warm[:],
                                     start=True, stop=True)

            for m, c0, c1, copies in sched:
                W = c1 - c0
                banks = (W + 511) // 512
                psb = ppool.tile([128, 512 * banks], dt32, tag=f"b{banks}",
                                 bufs=ps_bufs[512 * banks], name="psb")
                ps = psb[:, :W]
                n_kc = (K + 127) // 128
                for j in range(0, W, 512):
                    w = min(512, W - j)
                    for kc in range(n_kc):
                        k0, k1 = kc * 128, min(K, (kc + 1) * 128)
                        nc.tensor.matmul(
                            ps[:, j:j + w],
                            fact_sb[k0:k1, m * 128:(m + 1) * 128],
                            fact_sb[k0:k1, _ROWS_PER_CORE + c0 + j:
                                    _ROWS_PER_CORE + c0 + j + w],
                            start=(kc == 0), stop=(kc == n_kc - 1),
                        )
                x = 0
                for w, eng in copies:
                    ot = stage[m][:, c0 + x:c0 + x + w]
                    if eng == "act":
                        nc.scalar.copy(ot, ps[:, x:x + w])
                    else:
                        nc.vector.tensor_copy(ot, ps[:, x:x + w])
                    x += w
                assert x == W
                if pool_dma and (m, c0) in pool_dma:
                    nc.gpsimd.dma_start(out[m * 128:(m + 1) * 128, c0:c1],
                                        stage[m][:, c0:c1])
                else:
                    nc.sync.dma_start(out[m * 128:(m + 1) * 128, c0:c1],
                                      stage[m][:, c0:c1])
    nc.compile()
    return nc


def _get_nc(K):
    if K not in _COMPILED:
        _COMPILED[K] = _build_nc2(K)
    return _COMPILED[K]


def _pack_factors(A, B):
    """f64 factors (R, 4096) -> bf16 K-stacked factors (2R, 4096).

    "split2": A is represented to ~f32 accuracy as Ah + Al (hi/lo bf16
    pair); B is plain bf16.  The rank-2R contraction computes
    (Ah + Al) @ bf16(B), so the end-to-end error is just the bf16
    rounding of B plus the bf16 rounding of the OUTPUT (~2.5e-3 rel
    total) -- far inside the 2e-2 gate.  K does not affect TensorE time,
    but smaller K shrinks the input DMA."""
    import ml_dtypes
    bf = ml_dtypes.bfloat16
    Ah = A.astype(bf)
    Al = (A - Ah.astype(np.float64)).astype(bf)
    Bh = B.astype(bf)
    Ap = np.concatenate([Ah, Al], axis=0)
    Bp = np.concatenate([Bh, Bh], axis=0)
    return Ap, Bp


def _make_in_maps(Ap, Bp):
    """Pack per-core inputs: fact = [lhsT shard | rhs] (K, 4608) bf16."""
    in_maps = []
    for c in range(_N_CORES):
        shard = Ap[:, c * _ROWS_PER_CORE:(c + 1) * _ROWS_PER_CORE]
        fact = np.concatenate([shard, Bp], axis=1)
        in_maps.append({"fact": np.ascontiguousarray(fact)})
    return in_maps


def _run_device(A, B, trace=False):
    """A, B: (R, 4096) float64 factors.  Returns (psi_flat f32, results)."""
    from concourse.bass_utils import run_bass_kernel_spmd

    Ap, Bp = _pack_factors(A, B)
    nc = _get_nc(Ap.shape[0])
    in_maps = _make_in_maps(Ap, Bp)
    res = run_bass_kernel_spmd(
        nc, in_maps, core_ids=list(range(_N_CORES)), trace=trace
    )
    flat = np.concatenate(
        [r["out"].astype(np.float32).reshape(-1) for r in res.results]
    )
    return flat, res


def kernel(states, gates, gate_wires):
    fact = _build_factors(states, gates, gate_wires)
    # 2R rows must fit the 128-partition SBUF input tile; exotic gate lists
    # that blow up the cut rank take the exact dense path instead.
    if fact is None or 2 * fact[0].shape[0] > 128:
        return _dense_fallback(states, gates, gate_wires)
    A, B = fact
    flat, _ = _run_device(A, B)
    return flat.reshape((2,) * _N_WIRES)


# revision 37
# speedup vs baseline: 1.0035x; 1.0006x over previous
"""Trainium2 kernel for nn_Circuit_28123445854302.

24-wire statevector circuit (depth-4 brickwork, 46 two-qubit gates) applied to
a product state.  Strategy:

The statevector is sharded over its 3 leading wire axes across the 8 cores
(state-index sharding, as hinted).  Rather than streaming the 64 MB state
through every gate, we exploit the circuit's 1-D locality: across the middle
wire cut (12|12) only the gates that straddle the cut can raise the Schmidt
rank, so the final state factors EXACTLY as

    psi[left, right] = sum_r A[r, left] * B[r, right]        (rank R, tiny)

For the brickwork circuit R == 16.  A and B (R x 4096) are computed exactly on
the host in float64 with negligible cost (all tensors are O(R * 2^12)); every
element of the 2^24 statevector is then produced ON DEVICE by a K=R matmul:

    core c:  out[512, 4096] = A[:, c*512:(c+1)*512].T @ B        (rows = left
             indices with leading-3-wire bits == c, i.e. the core's shard)

Each core writes its contiguous shard of the output; the host gather is a
plain concatenate.

Device pipeline (cost-model-guided):
  * The kernel is output-DMA bound: the out shard per core is the only large
    HBM traffic.  Emitting it in bfloat16 (upcast to f32 on the host during
    the gather) halves that traffic; the bf16 rounding of the OUTPUT is the
    only error introduced (~1e-3 rel, vs the 2e-2 gate).
  * The matmul keeps A to ~f32 accuracy via a "split2" trick: A is split as
    hi=bf16(A), lo=bf16(A-hi) and the rank-R contraction is widened to 2R
    rows so one bf16 matmul accumulates (hi+lo) @ bf16(B) in fp32 PSUM.
    K does not affect TensorE time (cost = N output cycles), so the extra
    rows only cost input-DMA bytes.
  * Column-group software pipeline: matmul (PE) -> PSUM->SBUF cast-copy
    (alternating ScalarE/VectorE so neither is the straggler) -> streaming
    HWDGE DMA of each bf16 group to HBM.  Groups are sized small at the head
    (to start the output-DMA chain early) and 1024 wide in steady state
    (DMA transfer per group ~= copy time of the idle engine).

If a (hypothetical) non-local gate list makes the cut rank explode, we fall
back to an exact dense numpy simulation (same semantics as the reference).
"""

import numpy as np

_N_WIRES = 24
_CUT = 12
_HALF = 1 << _CUT          # 4096
_N_CORES = 8
_ROWS_PER_CORE = _HALF // _N_CORES   # 512
_MAX_RANK = 512


# ----------------------------------------------------------------------------
# Host-side exact middle-cut factorization (all tiny tensors, float64)
# ----------------------------------------------------------------------------

def _apply_2q(M, g, w0, w1, nloc):
    """Apply gate g[i0,o0,i1,o1] on local wires w0,w1 of every row of
    M (R, 2**nloc).  Matches reference: tensordot + moveaxis."""
    R = M.shape[0]
    T = M.reshape((R,) + (2,) * nloc)
    src = [4] + [0 if k == w0 else (2 if k == w1 else 5 + k) for k in range(nloc)]
    dst = [4] + [1 if k == w0 else (3 if k == w1 else 5 + k) for k in range(nloc)]
    return np.einsum(g, [0, 1, 2, 3], T, src, dst).reshape(R, -1)


def _apply_1q(M, P, w, nloc):
    """Apply P[i,o] on local wire w of every row of M (R, 2**nloc)."""
    R = M.shape[0]
    T = M.reshape((R,) + (2,) * nloc)
    src = [4] + [0 if t == w else 5 + t for t in range(nloc)]
    dst = [4] + [1 if t == w else 5 + t for t in range(nloc)]
    return np.einsum(P, [0, 1], T, src, dst).reshape(R, -1)


def _build_factors(states, gates, gate_wires):
    """psi = A.T @ B with A, B (R, 4096) float64, or None if rank > _MAX_RANK."""
    states = np.asarray(states, dtype=np.float64)
    gates = np.asarray(gates, dtype=np.float64)
    wires = np.asarray(gate_wires)
    NR = _N_WIRES - _CUT

    def outer(lo, hi):
        v = states[lo]
        for w in range(lo + 1, hi):
            v = np.kron(v, states[w])
        return v

    A = outer(0, _CUT)[None, :].copy()
    B = outer(_CUT, _N_WIRES)[None, :].copy()

    for gi in range(gates.shape[0]):
        w0, w1 = int(wires[gi, 0]), int(wires[gi, 1])
        g = gates[gi]
        if w0 == w1:
            return None  # ill-defined for the reference too; bail out
        if w0 > w1:
            g = np.transpose(g, (2, 3, 0, 1))
            w0, w1 = w1, w0
        if w1 < _CUT:
            A = _apply_2q(A, g, w0, w1, _CUT)
        elif w0 >= _CUT:
            B = _apply_2q(B, g, w0 - _CUT, w1 - _CUT, NR)
        else:
            # Gate straddles the cut: operator-Schmidt split (rank <= 4).
            M4 = g.reshape(4, 4)  # rows (i0,o0) act left, cols (i1,o1) act right
            U, s, Vt = np.linalg.svd(M4)
            rank = max(1, int((s > s[0] * 1e-14).sum()))
            newA, newB = [], []
            for k in range(rank):
                P = (U[:, k] * s[k]).reshape(2, 2)
                Q = Vt[k].reshape(2, 2)
                newA.append(_apply_1q(A, P, w0, _CUT))
                newB.append(_apply_1q(B, Q, w1 - _CUT, NR))
            A = np.concatenate(newA, 0)
            B = np.concatenate(newB, 0)
            # Exact recompression (drops only numerically-zero directions).
            if A.shape[0] > 4:
                qa, ra = np.linalg.qr(A.T)
                qb, rb = np.linalg.qr(B.T)
                u, sv, vt = np.linalg.svd(ra @ rb.T)
                keep = max(1, int((sv > (sv[0] if sv.size else 1.0) * 1e-13).sum()))
                A = (qa @ (u[:, :keep] * sv[:keep])).T
                B = vt[:keep] @ qb.T
            if A.shape[0] > _MAX_RANK:
                return None
    return A, B


# ----------------------------------------------------------------------------
# Dense fallback (exact reference semantics in numpy) — only used if the gate
# list is so non-local that the middle-cut rank explodes.
# ----------------------------------------------------------------------------

def _dense_fallback(states, gates, gate_wires):
    states = np.asarray(states, dtype=np.float32)
    gates = np.asarray(gates, dtype=np.float32)
    wires = np.asarray(gate_wires)
    psi = states[0]
    for w in range(1, _N_WIRES):
        psi = np.multiply.outer(psi, states[w])
    for g in range(gates.shape[0]):
        w0, w1 = int(wires[g, 0]), int(wires[g, 1])
        psi = np.tensordot(gates[g], psi, axes=[[0, 2], [w0, w1]])
        psi = np.moveaxis(psi, (0, 1), (w0, w1))
    return psi


# ----------------------------------------------------------------------------
# Device kernel: out[512, 4096] (bf16) = lhsT.T @ rhs  per core, pipelined
# ----------------------------------------------------------------------------

_COMPILED = {}


_HEAD_WIDTHS = [128, 384, 576, 960, 1024, 1024]
_WARMUP_MMS = 0


def _default_schedule(head=None, split=None, split_m0_only=False):
    """List of (m, c0, c1, copies) groups in issue order; `copies` is a list
    of (width, engine) sub-copies covering [c0, c1) and each group gets one
    output DMA (waiting on all its sub-copies via subtile deps).

    Small head groups prime the output-DMA chain; 1024-wide steady-state
    groups keep per-instruction overheads amortized.  DVE goes first: the
    scheduler estimates ScalarE's first copy late (one-time activation-table
    load), which would head-of-line-block the first output DMA on the SP
    sequencer."""
    sched = []
    engs = ["dve", "act"]
    gi = 0
    for m in range(_ROWS_PER_CORE // 128):
        widths = (head or _HEAD_WIDTHS) if m == 0 else [1024, 1024, 1024, 1024]
        c = 0
        for w in widths:
            if split and w >= 512 and (not split_m0_only or m == 0):
                # parallel split across both engines, sized to their speeds
                wa = min(w - 64, max(64, int(round(w * split / 64)) * 64))
                copies = [(wa, "act"), (w - wa, "dve")]
            else:
                copies = [(w, engs[gi % 2])]
            sched.append((m, c, c + w, copies))
            c += w
            gi += 1
        assert c == _HALF
    return sched


def _build_nc2(K, in_chunks=3, head=None, warmup=None, split=None,
               pool_dma=None, split_m0_only=False, in_splits=None,
               sched=None, ps_bufs=None):
    """Bass module: fact [K, 512+4096] bf16 -> out [512, 4096] bf16.

    fact columns 0:512 hold this core's lhsT block (A columns of its row
    shard); columns 512: hold rhs (= B, shared by all cores).
    """
    import concourse.bass as bass
    import concourse.tile as tile
    from concourse import bacc, mybir

    nc = bacc.Bacc(
        "TRN2",
        target_bir_lowering=False,
        debug=False,
        enable_asserts=False,
        num_devices=_N_CORES,
    )
    dt32 = mybir.dt.float32
    bf16 = mybir.dt.bfloat16
    # Packed input layout [lhsT_m0 (128) | rhs (4096) | lhsT_m1..3 (384)]:
    # the first 256 columns are exactly what the first matmul needs, so a
    # tiny pilot DMA chunk starts the pipeline as early as possible, while
    # rhs stays contiguous for every group.
    CW = _ROWS_PER_CORE + _HALF   # 4608 packed input columns
    fact = nc.dram_tensor("fact", [K, CW], bf16, kind="ExternalInput").ap()
    out = nc.dram_tensor("out", [_ROWS_PER_CORE, _HALF], bf16,
                         kind="ExternalOutput").ap()

    def lhsT_cols(m):
        return (0, 128) if m == 0 else \
            (128 + _HALF + (m - 1) * 128, 128 + _HALF + m * 128)

    if sched is None:
        sched = _default_schedule(head, split, split_m0_only)
    if warmup is None:
        warmup = _WARMUP_MMS
    # PSUM budget: 8 banks of 2 KB/partition.  Groups <=512 wide share the
    # one-bank "b1" tag (2 bufs); 1024-wide groups use two-bank tiles
    # (3 bufs) -> 2 + 6 = 8 banks.
    if ps_bufs is None:
        ps_bufs = {512: 2, 1024: 3}

    with tile.TileContext(nc) as tc:
        with (
            tc.tile_pool(name="const", bufs=1) as cpool,
            tc.tile_pool(name="ps", bufs=1, space=bass.MemorySpace.PSUM) as ppool,
            tc.tile_pool(name="outs", bufs=1) as opool,
        ):
            fact_sb = cpool.tile([K, CW], bf16)
            # Head chunk covers all lhsT columns + the first rhs group(s) so
            # the pipeline can start as soon as it lands; the rest streams in
            # behind it.
            if in_splits is not None:
                splits = list(in_splits)
            else:
                splits = [0, 640, 2560, 3584, CW]
            for a, b in zip(splits[:-1], splits[1:]):
                nc.sync.dma_start(fact_sb[:, a:b], fact[:, a:b])

            # Static staging: one full-width bf16 tile per m-chunk (32 KB per
            # partition total) so no copy ever waits on an output-DMA
            # completion to recycle a buffer.
            stage = [
                opool.tile([128, _HALF], bf16, tag=f"m{m}", bufs=1,
                           name=f"stage{m}")
                for m in range(_ROWS_PER_CORE // 128)
            ]

            if warmup:
                # Ramp the TensorE p-state during the input-DMA latency with
                # dummy matmuls on a zeroed scratch tile (results discarded;
                # the first real matmul overwrites the bank with start=True).
                warm = cpool.tile([128, 256], bf16, tag="warm", bufs=1,
                                  name="warm")
                nc.any.memset(warm[:], 0)
                wps = ppool.tile([128, 512], dt32, tag="b1",
                                 bufs=ps_bufs[512], name="wps")
                for _ in range(warmup):
                    nc.tensor.matmul(wps[:, :256], warm[:, :128], warm[:],
                                     start=True, stop=True)

            for m, c0, c1, copies in sched:
                W = c1 - c0
                banks = (W + 511) // 512
                psb = ppool.tile([128, 512 * banks], dt32, tag=f"b{banks}",
                                 bufs=ps_bufs[512 * banks], name="psb")
                ps = psb[:, :W]
                n_kc = (K + 127) // 128
                la, lb = lhsT_cols(m)
                for j in range(0, W, 512):
                    w = min(512, W - j)
                    for kc in range(n_kc):
                        k0, k1 = kc * 128, min(K, (kc + 1) * 128)
                        nc.tensor.matmul(
                            ps[:, j:j + w],
                            fact_sb[k0:k1, la:lb],
                            fact_sb[k0:k1, 128 + c0 + j:128 + c0 + j + w],
                            start=(kc == 0), stop=(kc == n_kc - 1),
                        )
                x = 0
                for w, eng in copies:
                    ot = stage[m][:, c0 + x:c0 + x + w]
                    if eng == "act":
                        nc.scalar.copy(ot, ps[:, x:x + w])
                    else:
                        nc.vector.tensor_copy(ot, ps[:, x:x + w])
                    x += w
                assert x == W
                if pool_dma and (m, c0) in pool_dma:
                    nc.gpsimd.dma_start(out[m * 128:(m + 1) * 128, c0:c1],
                                        stage[m][:, c0:c1])
                else:
                    nc.sync.dma_start(out[m * 128:(m + 1) * 128, c0:c1],
                                      stage[m][:, c0:c1])
    nc.compile()
    return nc


def _get_nc(K):
    if K not in _COMPILED:
        _COMPILED[K] = _build_nc2(K)
    return _COMPILED[K]


def _pack_factors(A, B):
    """f64 factors (R, 4096) -> bf16 K-stacked factors (2R, 4096).

    "split2": A is represented to ~f32 accuracy as Ah + Al (hi/lo bf16
    pair); B is plain bf16.  The rank-2R contraction computes
    (Ah + Al) @ bf16(B), so the end-to-end error is just the bf16
    rounding of B plus the bf16 rounding of the OUTPUT (~2.5e-3 rel
    total) -- far inside the 2e-2 gate.  K does not affect TensorE time,
    but smaller K shrinks the input DMA."""
    import ml_dtypes
    bf = ml_dtypes.bfloat16
    Ah = A.astype(bf)
    Al = (A - Ah.astype(np.float64)).astype(bf)
    Bh = B.astype(bf)
    Ap = np.concatenate([Ah, Al], axis=0)
    Bp = np.concatenate([Bh, Bh], axis=0)
    return Ap, Bp


def _make_in_maps(Ap, Bp):
    """Pack per-core inputs: fact = [lhsT_m0 | rhs | lhsT_m1..3] (K, 4608)
    bf16 (see _build_nc2: the first 256 columns feed the first matmul)."""
    in_maps = []
    for c in range(_N_CORES):
        shard = Ap[:, c * _ROWS_PER_CORE:(c + 1) * _ROWS_PER_CORE]
        fact = np.concatenate([shard[:, :128], Bp, shard[:, 128:]], axis=1)
        in_maps.append({"fact": np.ascontiguousarray(fact)})
    return in_maps


def _run_device(A, B, trace=False):
    """A, B: (R, 4096) float64 factors.  Returns (psi_flat f32, results)."""
    from concourse.bass_utils import run_bass_kernel_spmd

    Ap, Bp = _pack_factors(A, B)
    nc = _get_nc(Ap.shape[0])
    in_maps = _make_in_maps(Ap, Bp)
    res = run_bass_kernel_spmd(
        nc, in_maps, core_ids=list(range(_N_CORES)), trace=trace
    )
    flat = np.concatenate(
        [r["out"].astype(np.float32).reshape(-1) for r in res.results]
    )
    return flat, res


def kernel(states, gates, gate_wires):
    fact = _build_factors(states, gates, gate_wires)
    # 2R rows must fit the 128-partition SBUF input tile; exotic gate lists
    # that blow up the cut rank take the exact dense path instead.
    if fact is None or 2 * fact[0].shape[0] > 128:
        return _dense_fallback(states, gates, gate_wires)
    A, B = fact
    flat, _ = _run_device(A, B)
    return flat.reshape((2,) * _N_WIRES)


# revision 38
# speedup vs baseline: 1.0043x; 1.0008x over previous
"""Trainium2 kernel for nn_Circuit_28123445854302.

24-wire statevector circuit (depth-4 brickwork, 46 two-qubit gates) applied to
a product state.  Strategy:

The statevector is sharded over its 3 leading wire axes across the 8 cores
(state-index sharding, as hinted).  Rather than streaming the 64 MB state
through every gate, we exploit the circuit's 1-D locality: across the middle
wire cut (12|12) only the gates that straddle the cut can raise the Schmidt
rank, so the final state factors EXACTLY as

    psi[left, right] = sum_r A[r, left] * B[r, right]        (rank R, tiny)

For the brickwork circuit R == 16.  A and B (R x 4096) are computed exactly on
the host in float64 with negligible cost (all tensors are O(R * 2^12)); every
element of the 2^24 statevector is then produced ON DEVICE by a K=R matmul:

    core c:  out[512, 4096] = A[:, c*512:(c+1)*512].T @ B        (rows = left
             indices with leading-3-wire bits == c, i.e. the core's shard)

Each core writes its contiguous shard of the output; the host gather is a
plain concatenate.

Device pipeline (cost-model-guided):
  * The kernel is output-DMA bound: the out shard per core is the only large
    HBM traffic.  Emitting it in bfloat16 (upcast to f32 on the host during
    the gather) halves that traffic; the bf16 rounding of the OUTPUT is the
    only error introduced (~1e-3 rel, vs the 2e-2 gate).
  * The matmul keeps A to ~f32 accuracy via a "split2" trick: A is split as
    hi=bf16(A), lo=bf16(A-hi) and the rank-R contraction is widened to 2R
    rows so one bf16 matmul accumulates (hi+lo) @ bf16(B) in fp32 PSUM.
    K does not affect TensorE time (cost = N output cycles), so the extra
    rows only cost input-DMA bytes.
  * Column-group software pipeline: matmul (PE) -> PSUM->SBUF cast-copy
    (alternating ScalarE/VectorE so neither is the straggler) -> streaming
    HWDGE DMA of each bf16 group to HBM.  Groups are sized small at the head
    (to start the output-DMA chain early) and 1024 wide in steady state
    (DMA transfer per group ~= copy time of the idle engine).

If a (hypothetical) non-local gate list makes the cut rank explode, we fall
back to an exact dense numpy simulation (same semantics as the reference).
"""

import numpy as np

_N_WIRES = 24
_CUT = 12
_HALF = 1 << _CUT          # 4096
_N_CORES = 8
_ROWS_PER_CORE = _HALF // _N_CORES   # 512
_MAX_RANK = 512


# ----------------------------------------------------------------------------
# Host-side exact middle-cut factorization (all tiny tensors, float64)
# ----------------------------------------------------------------------------

def _apply_2q(M, g, w0, w1, nloc):
    """Apply gate g[i0,o0,i1,o1] on local wires w0,w1 of every row of
    M (R, 2**nloc).  Matches reference: tensordot + moveaxis."""
    R = M.shape[0]
    T = M.reshape((R,) + (2,) * nloc)
    src = [4] + [0 if k == w0 else (2 if k == w1 else 5 + k) for k in range(nloc)]
    dst = [4] + [1 if k == w0 else (3 if k == w1 else 5 + k) for k in range(nloc)]
    return np.einsum(g, [0, 1, 2, 3], T, src, dst).reshape(R, -1)


def _apply_1q(M, P, w, nloc):
    """Apply P[i,o] on local wire w of every row of M (R, 2**nloc)."""
    R = M.shape[0]
    T = M.reshape((R,) + (2,) * nloc)
    src = [4] + [0 if t == w else 5 + t for t in range(nloc)]
    dst = [4] + [1 if t == w else 5 + t for t in range(nloc)]
    return np.einsum(P, [0, 1], T, src, dst).reshape(R, -1)


def _build_factors(states, gates, gate_wires):
    """psi = A.T @ B with A, B (R, 4096) float64, or None if rank > _MAX_RANK."""
    states = np.asarray(states, dtype=np.float64)
    gates = np.asarray(gates, dtype=np.float64)
    wires = np.asarray(gate_wires)
    NR = _N_WIRES - _CUT

    def outer(lo, hi):
        v = states[lo]
        for w in range(lo + 1, hi):
            v = np.kron(v, states[w])
        return v

    A = outer(0, _CUT)[None, :].copy()
    B = outer(_CUT, _N_WIRES)[None, :].copy()

    for gi in range(gates.shape[0]):
        w0, w1 = int(wires[gi, 0]), int(wires[gi, 1])
        g = gates[gi]
        if w0 == w1:
            return None  # ill-defined for the reference too; bail out
        if w0 > w1:
            g = np.transpose(g, (2, 3, 0, 1))
            w0, w1 = w1, w0
        if w1 < _CUT:
            A = _apply_2q(A, g, w0, w1, _CUT)
        elif w0 >= _CUT:
            B = _apply_2q(B, g, w0 - _CUT, w1 - _CUT, NR)
        else:
            # Gate straddles the cut: operator-Schmidt split (rank <= 4).
            M4 = g.reshape(4, 4)  # rows (i0,o0) act left, cols (i1,o1) act right
            U, s, Vt = np.linalg.svd(M4)
            rank = max(1, int((s > s[0] * 1e-14).sum()))
            newA, newB = [], []
            for k in range(rank):
                P = (U[:, k] * s[k]).reshape(2, 2)
                Q = Vt[k].reshape(2, 2)
                newA.append(_apply_1q(A, P, w0, _CUT))
                newB.append(_apply_1q(B, Q, w1 - _CUT, NR))
            A = np.concatenate(newA, 0)
            B = np.concatenate(newB, 0)
            # Exact recompression (drops only numerically-zero directions).
            if A.shape[0] > 4:
                qa, ra = np.linalg.qr(A.T)
                qb, rb = np.linalg.qr(B.T)
                u, sv, vt = np.linalg.svd(ra @ rb.T)
                keep = max(1, int((sv > (sv[0] if sv.size else 1.0) * 1e-13).sum()))
                A = (qa @ (u[:, :keep] * sv[:keep])).T
                B = vt[:keep] @ qb.T
            if A.shape[0] > _MAX_RANK:
                return None
    return A, B


# ----------------------------------------------------------------------------
# Dense fallback (exact reference semantics in numpy) — only used if the gate
# list is so non-local that the middle-cut rank explodes.
# ----------------------------------------------------------------------------

def _dense_fallback(states, gates, gate_wires):
    states = np.asarray(states, dtype=np.float32)
    gates = np.asarray(gates, dtype=np.float32)
    wires = np.asarray(gate_wires)
    psi = states[0]
    for w in range(1, _N_WIRES):
        psi = np.multiply.outer(psi, states[w])
    for g in range(gates.shape[0]):
        w0, w1 = int(wires[g, 0]), int(wires[g, 1])
        psi = np.tensordot(gates[g], psi, axes=[[0, 2], [w0, w1]])
        psi = np.moveaxis(psi, (0, 1), (w0, w1))
    return psi


# ----------------------------------------------------------------------------
# Device kernel: out[512, 4096] (bf16) = lhsT.T @ rhs  per core, pipelined
# ----------------------------------------------------------------------------

_COMPILED = {}


_HEAD_WIDTHS = [128, 384, 576, 1024, 960, 1024]
_WARMUP_MMS = 0


def _default_schedule(head=None, split=None, split_m0_only=False):
    """List of (m, c0, c1, copies) groups in issue order; `copies` is a list
    of (width, engine) sub-copies covering [c0, c1) and each group gets one
    output DMA (waiting on all its sub-copies via subtile deps).

    Small head groups prime the output-DMA chain; 1024-wide steady-state
    groups keep per-instruction overheads amortized.  DVE goes first: the
    scheduler estimates ScalarE's first copy late (one-time activation-table
    load), which would head-of-line-block the first output DMA on the SP
    sequencer."""
    sched = []
    engs = ["dve", "act"]
    gi = 0
    for m in range(_ROWS_PER_CORE // 128):
        widths = (head or _HEAD_WIDTHS) if m == 0 else [1024, 1024, 1024, 1024]
        c = 0
        for w in widths:
            if split and w >= 512 and (not split_m0_only or m == 0):
                # parallel split across both engines, sized to their speeds
                wa = min(w - 64, max(64, int(round(w * split / 64)) * 64))
                copies = [(wa, "act"), (w - wa, "dve")]
            else:
                copies = [(w, engs[gi % 2])]
            sched.append((m, c, c + w, copies))
            c += w
            gi += 1
        assert c == _HALF
    return sched


def _build_nc2(K, in_chunks=3, head=None, warmup=None, split=None,
               pool_dma=None, split_m0_only=False, in_splits=None,
               sched=None, ps_bufs=None):
    """Bass module: fact [K, 512+4096] bf16 -> out [512, 4096] bf16.

    fact columns 0:512 hold this core's lhsT block (A columns of its row
    shard); columns 512: hold rhs (= B, shared by all cores).
    """
    import concourse.bass as bass
    import concourse.tile as tile
    from concourse import bacc, mybir

    nc = bacc.Bacc(
        "TRN2",
        target_bir_lowering=False,
        debug=False,
        enable_asserts=False,
        num_devices=_N_CORES,
    )
    dt32 = mybir.dt.float32
    bf16 = mybir.dt.bfloat16
    # Packed input layout [lhsT_m0 (128) | rhs (4096) | lhsT_m1..3 (384)]:
    # the first 256 columns are exactly what the first matmul needs, so a
    # tiny pilot DMA chunk starts the pipeline as early as possible, while
    # rhs stays contiguous for every group.
    CW = _ROWS_PER_CORE + _HALF   # 4608 packed input columns
    fact = nc.dram_tensor("fact", [K, CW], bf16, kind="ExternalInput").ap()
    out = nc.dram_tensor("out", [_ROWS_PER_CORE, _HALF], bf16,
                         kind="ExternalOutput").ap()

    def lhsT_cols(m):
        return (0, 128) if m == 0 else \
            (128 + _HALF + (m - 1) * 128, 128 + _HALF + m * 128)

    if sched is None:
        sched = _default_schedule(head, split, split_m0_only)
    if warmup is None:
        warmup = _WARMUP_MMS
    # PSUM budget: 8 banks of 2 KB/partition.  Groups <=512 wide share the
    # one-bank "b1" tag (2 bufs); 1024-wide groups use two-bank tiles
    # (3 bufs) -> 2 + 6 = 8 banks.
    if ps_bufs is None:
        ps_bufs = {512: 2, 1024: 3}

    with tile.TileContext(nc) as tc:
        with (
            tc.tile_pool(name="const", bufs=1) as cpool,
            tc.tile_pool(name="ps", bufs=1, space=bass.MemorySpace.PSUM) as ppool,
            tc.tile_pool(name="outs", bufs=1) as opool,
        ):
            fact_sb = cpool.tile([K, CW], bf16)
            # Head chunk covers all lhsT columns + the first rhs group(s) so
            # the pipeline can start as soon as it lands; the rest streams in
            # behind it.
            if in_splits is not None:
                splits = list(in_splits)
            else:
                splits = [0, 640, 2560, 3584, CW]
            for a, b in zip(splits[:-1], splits[1:]):
                nc.sync.dma_start(fact_sb[:, a:b], fact[:, a:b])

            # Static staging: one full-width bf16 tile per m-chunk (32 KB per
            # partition total) so no copy ever waits on an output-DMA
            # completion to recycle a buffer.
            stage = [
                opool.tile([128, _HALF], bf16, tag=f"m{m}", bufs=1,
                           name=f"stage{m}")
                for m in range(_ROWS_PER_CORE // 128)
            ]

            if warmup:
                # Ramp the TensorE p-state during the input-DMA latency with
                # dummy matmuls on a zeroed scratch tile (results discarded;
                # the first real matmul overwrites the bank with start=True).
                warm = cpool.tile([128, 256], bf16, tag="warm", bufs=1,
                                  name="warm")
                nc.any.memset(warm[:], 0)
                wps = ppool.tile([128, 512], dt32, tag="b1",
                                 bufs=ps_bufs[512], name="wps")
                for _ in range(warmup):
                    nc.tensor.matmul(wps[:, :256], warm[:, :128], warm[:],
                                     start=True, stop=True)

            for m, c0, c1, copies in sched:
                W = c1 - c0
                banks = (W + 511) // 512
                psb = ppool.tile([128, 512 * banks], dt32, tag=f"b{banks}",
                                 bufs=ps_bufs[512 * banks], name="psb")
                ps = psb[:, :W]
                n_kc = (K + 127) // 128
                la, lb = lhsT_cols(m)
                for j in range(0, W, 512):
                    w = min(512, W - j)
                    for kc in range(n_kc):
                        k0, k1 = kc * 128, min(K, (kc + 1) * 128)
                        nc.tensor.matmul(
                            ps[:, j:j + w],
                            fact_sb[k0:k1, la:lb],
                            fact_sb[k0:k1, 128 + c0 + j:128 + c0 + j + w],
                            start=(kc == 0), stop=(kc == n_kc - 1),
                        )
                x = 0
                for w, eng in copies:
                    ot = stage[m][:, c0 + x:c0 + x + w]
                    if eng == "act":
                        nc.scalar.copy(ot, ps[:, x:x + w])
                    else:
                        nc.vector.tensor_copy(ot, ps[:, x:x + w])
                    x += w
                assert x == W
                if pool_dma and (m, c0) in pool_dma:
                    nc.gpsimd.dma_start(out[m * 128:(m + 1) * 128, c0:c1],
                                        stage[m][:, c0:c1])
                else:
                    nc.sync.dma_start(out[m * 128:(m + 1) * 128, c0:c1],
                                      stage[m][:, c0:c1])
    nc.compile()
    return nc


def _get_nc(K):
    if K not in _COMPILED:
        _COMPILED[K] = _build_nc2(K)
    return _COMPILED[K]


def _pack_factors(A, B):
    """f64 factors (R, 4096) -> bf16 K-stacked factors (2R, 4096).

    "split2": A is represented to ~f32 accuracy as Ah + Al (hi/lo bf16
    pair); B is plain bf16.  The rank-2R contraction computes
    (Ah + Al) @ bf16(B), so the end-to-end error is just the bf16
    rounding of B plus the bf16 rounding of the OUTPUT (~2.5e-3 rel
    total) -- far inside the 2e-2 gate.  K does not affect TensorE time,
    but smaller K shrinks the input DMA."""
    import ml_dtypes
    bf = ml_dtypes.bfloat16
    Ah = A.astype(bf)
    Al = (A - Ah.astype(np.float64)).astype(bf)
    Bh = B.astype(bf)
    Ap = np.concatenate([Ah, Al], axis=0)
    Bp = np.concatenate([Bh, Bh], axis=0)
    return Ap, Bp


def _make_in_maps(Ap, Bp):
    """Pack per-core inputs: fact = [lhsT_m0 | rhs | lhsT_m1..3] (K, 4608)
    bf16 (see _build_nc2: the first 256 columns feed the first matmul)."""
    in_maps = []
    for c in range(_N_CORES):
        shard = Ap[:, c * _ROWS_PER_CORE:(c + 1) * _ROWS_PER_CORE]
        fact = np.concatenate([shard[:, :128], Bp, shard[:, 128:]], axis=1)
        in_maps.append({"fact": np.ascontiguousarray(fact)})
    return in_maps


def _run_device(A, B, trace=False):
    """A, B: (R, 4096) float64 factors.  Returns (psi_flat f32, results)."""
    from concourse.bass_utils import run_bass_kernel_spmd

    Ap, Bp = _pack_factors(A, B)
    nc = _get_nc(Ap.shape[0])
    in_maps = _make_in_maps(Ap, Bp)
    res = run_bass_kernel_spmd(
        nc, in_maps, core_ids=list(range(_N_CORES)), trace=trace
    )
    flat = np.concatenate(
        [r["out"].astype(np.float32).reshape(-1) for r in res.results]
    )
    return flat, res


def kernel(states, gates, gate_wires):
    fact = _build_factors(states, gates, gate_wires)
    # 2R rows must fit the 128-partition SBUF input tile; exotic gate lists
    # that blow up the cut rank take the exact dense path instead.
    if fact is None or 2 * fact[0].shape[0] > 128:
        return _dense_fallback(states, gates, gate_wires)
    A, B = fact
    flat, _ = _run_device(A, B)
    return flat.reshape((2,) * _N_WIRES)
